# revision 26
# baseline (speedup 1.0000x reference)
"""Trainium2 Bass kernel for nn_Attention_21715354649378.

Reference computation (per batch b of 4):
    qkv = w_qkv @ x        x: [256, 4096(=64x64)]   w_qkv: [384, 256]
    q,k,v: [4 heads, 32, 4096];  q *= 32**-0.5
    sim_h = q_h^T k_h   [4096, 4096];  attn = softmax(sim, axis=-1)
    out_h = attn @ v_h^T    -> [4096, 32]
    out = w_out @ concat_heads + b_out   [256, 4096]

Sharding: 8 cores = 4 batches x 2 query-halves. Each core computes K/V for
its full batch plus attention + output projection for its half of the query
pixels. Outputs are disjoint slices -> no collectives.

Device algorithm per core (keys-in-partition layout; probs are kept in
fp8e4m3 with a global exp-shift of -2 which cancels in the softmax ratio):
    vT8 = x^T W_v^T      fp8 pair layout [128keys, pair, j, (h|1)x48]
    krep_h = repl4(W_k,h) x   [128 = 4 copies of k_h(32d), 4096]  bf16
    qrep_h = repl4(s W_q,h) xq [128, 2048] bf16
    flat software pipeline over chunks (h, ci) and key-tile groups, with two
    alternating PSUM staging pools (4 + 3 banks) shared with the projection
    stream; PV trails exp so activations run back-to-back:
        simT[kt] = krep_h[band, kt].T @ qrep_h[band, ci]   -> PSUM
        probs8 = exp(simT - 2)  -> fp8 ring  (ScalarE activation, or DVE
                 via Schraudolph: uint8(rint(x*a + b)) = e4m3 bits)
        pv += DoubleRow fp8 matmul over key-tile PAIRS:
              [vh|1](128,2,33).T @ probs8(128,2,512)  -> [33, 512]
    rows 0..31 = unnormalized out, row 32 = softmax denominator;
    outh[ci][32h:] = pv[0:32] * bcast(1/pv[32]) (recip + DRAM-bounce DMA)
    out[ci] = W_o @ outh[ci] + b_out  -> DMA out
"""

import numpy as np
import ml_dtypes

import concourse.bass as bass
import concourse.mybir as mybir
import concourse.tile as tile
from concourse import bacc
from concourse.bass import ts, ds
from concourse.bass_utils import run_bass_kernel_spmd

HEADS = 4
D = 32
HID = 128
C = 256
N = 4096
NQ = 2048
SCALE = D ** -0.5
NCORES = 8

F32 = mybir.dt.float32
F32R = mybir.dt.float32r
BF16 = mybir.dt.bfloat16
F8 = mybir.dt.float8e4
U8 = mybir.dt.uint8
EXP = mybir.ActivationFunctionType.Exp
DRMODE = mybir.MatmulPerfMode.DoubleRow

# exp shift: probs = exp(sim - C2); cancels in softmax normalization but
# keeps exp(sim) within fp8e4m3 range (max 240) for sim up to ~7.4
C2 = 2.0
# Schraudolph fp8e4m3: bits = rint(x*A8 + B8) as saturating uint8
A8 = 8.0 / np.log(2.0)
B8 = 56.0 - C2 * A8

NKT = N // 128  # 32 key tiles per chunk
NPAIR = NKT // 2  # 16 DoubleRow pairs
NCH = NQ // 512  # 4 query chunks
PVLAG = 8  # PV trails its exp by this many staging groups
VTW = 48  # padded per-head width in vT8 (33 used; 48 for 16B ldw stride)


def build_nc():
    nc = bacc.Bacc("TRN2")

    xb = nc.declare_dram_parameter("xb", [C, N], BF16, isOutput=False)
    xq = nc.declare_dram_parameter("xq", [C, NQ], BF16, isOutput=False)
    wqrT = nc.declare_dram_parameter("wqrT", [C, HEADS * HID], BF16, isOutput=False)
    wkrT = nc.declare_dram_parameter("wkrT", [C, HEADS * HID], BF16, isOutput=False)
    wvT = nc.declare_dram_parameter("wvT", [C, HID], BF16, isOutput=False)
    woT = nc.declare_dram_parameter("woT", [HID, C], F32R, isOutput=False)
    bout = nc.declare_dram_parameter("bout", [C, 1], F32, isOutput=False)
    out = nc.declare_dram_parameter("out", [C, NQ], F32, isOutput=True)

    with tile.TileContext(nc) as tc:
        with (
            nc.allow_low_precision(reason="bf16/fp8 attention core"),
            tc.tile_pool(name="persist", bufs=1) as persist,
            tc.tile_pool(name="wts", bufs=1) as wts,
            tc.tile_pool(name="dram", bufs=2, space="DRAM") as dram_pool,
        ):
            # ---- persistent SBUF tensors ----
            x_sb = [
                [
                    persist.tile([128, N // 2], BF16, tag=f"x{i}{j}", name=f"x{i}{j}")
                    for j in range(2)
                ]
                for i in range(2)
            ]
            xq_sb = [
                persist.tile([128, NQ], BF16, tag=f"xq{i}", name=f"xq{i}")
                for i in range(2)
            ]
            krep = [
                persist.tile([128, N], BF16, tag=f"krep{h}", name=f"krep{h}")
                for h in range(HEADS)
            ]
            qrep = [
                persist.tile([128, NQ], BF16, tag=f"qrep{h}", name=f"qrep{h}")
                for h in range(HEADS)
            ]
            # fp8 V in DoubleRow pair layout: [128keys, pair, j, h*VTW + m]
            # (m: 0..31 = v dims, 32 = ones for the softmax denominator)
            vt8 = persist.tile(
                [128, NPAIR, 2, HEADS * VTW], F8, tag="vt8", name="vt8"
            )

            wqr_sb = [
                wts.tile([128, HEADS * HID], BF16, tag=f"wqr{i}", name=f"wqr{i}")
                for i in range(2)
            ]
            wkr_sb = [
                wts.tile([128, HEADS * HID], BF16, tag=f"wkr{i}", name=f"wkr{i}")
                for i in range(2)
            ]
            wv_sb = [
                wts.tile([128, HID], BF16, tag=f"wv{i}", name=f"wv{i}")
                for i in range(2)
            ]
            wo_sb = wts.tile([HID, C], F32R, tag="wo")
            bo_sb = [
                wts.tile([128, 1], F32, tag=f"bo{i}", name=f"bo{i}")
                for i in range(2)
            ]
            ones_sb = wts.tile([1, D], F32, tag="ones")
            nbias_sb = wts.tile([128, 1], F32, tag="nbias")

            # ---- DMA inputs, ordered by first use (~0.6us issue each) ----
            for i in range(2):
                nc.sync.dma_start(out=wkr_sb[i][:], in_=wkrT[ds(i * 128, 128), :])
            for i in range(2):
                nc.sync.dma_start(
                    out=x_sb[i][0][:], in_=xb[ds(i * 128, 128), ts(0, N // 2)]
                )
            for i in range(2):
                nc.gpsimd.dma_start(out=wv_sb[i][:], in_=wvT[ds(i * 128, 128), :])
                nc.gpsimd.dma_start(out=wqr_sb[i][:], in_=wqrT[ds(i * 128, 128), :])
            for i in range(2):
                nc.gpsimd.dma_start(out=xq_sb[i][:], in_=xq[ds(i * 128, 128), :])
            for i in range(2):
                nc.sync.dma_start(
                    out=x_sb[i][1][:], in_=xb[ds(i * 128, 128), ts(1, N // 2)]
                )
                nc.gpsimd.dma_start(out=bo_sb[i][:], in_=bout[ds(i * 128, 128), :])
            nc.gpsimd.dma_start(out=wo_sb[:], in_=woT[:, :])
            nc.vector.memset(vt8[:], 1.0)
            nc.vector.memset(ones_sb[:], 1.0)
            nc.vector.memset(nbias_sb[:], -C2)

            with (
                tc.tile_pool(name="qkA", bufs=1, space="PSUM") as qkA,
                tc.tile_pool(name="qkB", bufs=1, space="PSUM") as qkB,
                tc.tile_pool(name="pvp", bufs=1, space="PSUM") as pvp,
                tc.tile_pool(name="ring", bufs=2) as ring_pool,
                tc.tile_pool(name="norm", bufs=3) as norm_pool,
                tc.tile_pool(name="osb", bufs=2) as osb,
            ):
                # staging slots rotate globally between the two pools;
                # projection tiles share the same rotation (no extra banks)
                _ptog = [0]

                def x_ap(ct, c0, length):
                    t_idx = c0 // (N // 2)
                    return x_sb[ct][t_idx][:, ds(c0 % (N // 2), length)]

                def xq_ap(ct, c0, length):
                    return xq_sb[ct][:, ds(c0, length)]

                def next_pool():
                    pool = qkA if _ptog[0] == 0 else qkB
                    _ptog[0] ^= 1
                    return pool

                def proj_tile(cols):
                    pool = next_pool()
                    t = pool.tile(
                        [128, (4 if pool is qkA else 3) * 512],
                        F32,
                        tag="qk",
                        name="ps",
                    )
                    return t[:, 0:cols]

                def emit_vt4(kt0):
                    # four key tiles' vT in one staging slot, one strided copy
                    ps = proj_tile(4 * HID)
                    for t in range(4):
                        for ct in range(2):
                            nc.tensor.matmul(
                                ps[:, ts(t, HID)],
                                x_ap(ct, (kt0 + t) * 128, 128),
                                wv_sb[ct][:],
                                start=(ct == 0),
                                stop=(ct == 1),
                            )
                    # dst: [p, pair(2), j(2), h(4), m(32)] slice of vt8
                    dst = vt8[:, ds(kt0 // 2, 2), :, :].rearrange(
                        "p t j (h m) -> p t j h m", m=VTW
                    )[:, :, :, :, 0:32]
                    src = ps.rearrange("p (t h m) -> p t h m", t=4, m=32).rearrange(
                        "p (tp j) h m -> p tp j h m", j=2
                    )
                    nc.scalar.copy(dst, src)

                def emit_k(h, j, nj=1):
                    # fill one staging slot with nj 512-col key-projection
                    # blocks (one matmul pair per block, a single evac CAST)
                    ps = proj_tile(nj * 512)
                    for b in range(nj):
                        for ct in range(2):
                            nc.tensor.matmul(
                                ps[:, ts(b, 512)],
                                wkr_sb[ct][:, ts(h, HID)],
                                x_ap(ct, (j + b) * 512, 512),
                                start=(ct == 0),
                                stop=(ct == 1),
                            )
                    nc.scalar.copy(krep[h][:, ds(j * 512, nj * 512)], ps[:])

                def emit_q(h, j, nj=1):
                    ps = proj_tile(nj * 512)
                    for b in range(nj):
                        for ct in range(2):
                            nc.tensor.matmul(
                                ps[:, ts(b, 512)],
                                wqr_sb[ct][:, ts(h, HID)],
                                xq_ap(ct, (j + b) * 512, 512),
                                start=(ct == 0),
                                stop=(ct == 1),
                            )
                    nc.scalar.copy(qrep[h][:, ds(j * 512, nj * 512)], ps[:])

                outh = [
                    osb.tile([HID, 512], F32R, tag=f"outh{c}", name=f"outh{c}")
                    for c in range(NCH)
                ]

                # one PSUM bank holds TWO pv accumulators (partitions 0 and
                # 64), alternating per (h,ci) so a chunk's first PV matmul
                # never stalls the in-order Tensor queue waiting for the
                # previous chunk's normalize to evacuate the bank.
                pv_bank = pvp.tile([128, 512], F32, tag="pv", name="pv")

                def emit_norm(h, ci, pv, tail=False):
                    # rows 0..31 / row 32
                    pvs = norm_pool.tile([33, 512], F32, tag="pvs", name="pvs")
                    nc.vector.tensor_copy(pvs[:], pv[0:33, :])
                    den = norm_pool.tile([1, 512], F32, tag="den", name="den")
                    nc.vector.tensor_copy(den[:], pv[32:33, :])
                    rec = norm_pool.tile([1, 512], F32, tag="rec", name="rec")
                    # (reciprocal_approx_fast requires a partition-0 source)
                    nc.vector.reciprocal_approx_fast(rec[:], den[:])
                    if tail:
                        # staging slots are free at the end: matmul-broadcast
                        # avoids the ~5us DRAM round-trip on the critical tail
                        bcp = next_pool()
                        bct = bcp.tile(
                            [128, (4 if bcp is qkA else 3) * 512],
                            F32,
                            tag="qk",
                            name="bct",
                        )
                        nc.tensor.matmul(
                            bct[0:D, 0:512],
                            ones_sb[:],
                            rec[:],
                            start=True,
                            stop=True,
                        )
                        nc.vector.tensor_mul(
                            outh[ci][ds(32 * h, 32), :],
                            pvs[0:32, :],
                            bct[0:D, 0:512],
                        )
                        return
                    # broadcast 1/denom to 32 partitions via DRAM bounce
                    rdr = dram_pool.tile([1, 512], F32, tag="rdr", name="rdr")
                    nc.sync.dma_start(out=rdr[:], in_=rec[:])
                    bc = norm_pool.tile([D, 512], F32, tag="bc", name="bc")
                    nc.sync.dma_start(
                        out=bc[:],
                        in_=bass.AP(
                            tensor=rdr.tensor,
                            offset=rdr.offset,
                            ap=[[0, D]] + [list(a) for a in rdr.ap[1:]],
                        ),
                    )
                    nc.gpsimd.tensor_mul(
                        outh[ci][ds(32 * h, 32), :], pvs[0:32, :], bc[:]
                    )

                pending = []
                deferred_op = []
                _gc = [0]

                def emit_outproj(ci):
                    for ot in range(2):
                        op = proj_tile(512)
                        nc.tensor.matmul(
                            op,
                            wo_sb[:, ts(ot, 128)],
                            outh[ci][:],
                            start=True,
                            stop=True,
                        )
                        ob = osb.tile([128, 512], F32, tag="ob", name="ob")
                        nc.vector.tensor_scalar_add(ob[:], op, bo_sb[ot][:])
                        nc.gpsimd.dma_start(
                            out=out[ds(ot * 128, 128), ts(ci, 512)], in_=ob[:]
                        )

                # per-(h,ci) watermark of PV'd key tiles (pairwise DoubleRow)
                pv_done = {}

                def pop_pv():
                    ring, kt0, gsz, h, ci, pv, pb = pending.pop(0)
                    wm = kt0 + gsz
                    done = pv_done.get((h, ci), 0)
                    while done + 2 <= wm:
                        t = done // 2
                        lhsT = vt8[:, ds(t, 1), :, ds(h * VTW, 33)].rearrange(
                            "p a j m -> p (a j) m"
                        )
                        rhs = ring[:, ds(2 * t, 2), :]
                        nc.tensor.matmul(
                            pv,
                            lhsT,
                            rhs,
                            start=(t == 0),
                            stop=(t == NPAIR - 1),
                            perf_mode=DRMODE,
                        )
                        done += 2
                    pv_done[(h, ci)] = done
                    if done == NKT:
                        last = h == HEADS - 1 and ci == NCH - 1
                        emit_norm(h, ci, pv, tail=last)
                        if h == HEADS - 1:
                            deferred_op.append(ci)

                # prologue: first projections
                emit_k(0, 0, 2)
                emit_vt4(0)
                emit_q(0, 0)

                for h in range(HEADS):
                    for ci in range(NCH):
                        pb = 0
                        pv = pv_bank[ds(pb, 33), :]
                        ring = ring_pool.tile(
                            [128, NKT, 512], F8, tag="ring", name="ring"
                        )
                        kt = 0
                        g = -2
                        while kt < NKT:
                            g += 2
                            # pair of QK groups back-to-back: a full<->tiled
                            # PE mode switch drains the array, so batching
                            # two row-banded QK groups (then two PV groups)
                            # halves the switches and keeps QKs concurrent
                            qks = []
                            for _ in range(2):
                                if kt >= NKT:
                                    break
                                pool = next_pool()
                                gsz = min(4 if pool is qkA else 3, NKT - kt)
                                qk = pool.tile(
                                    [128, gsz * 512], F32, tag="qk", name="qkg"
                                )
                                for j in range(gsz):
                                    band = (kt + j) % 4
                                    nc.tensor.matmul(
                                        qk[:, ts(j, 512)],
                                        krep[h][ds(32 * band, 32), ts(kt + j, 128)],
                                        qrep[h][ds(32 * band, 32), ts(ci, 512)],
                                        start=True,
                                        stop=True,
                                        tile_position=(32 * band, 0),
                                    )
                                qks.append((qk, kt, gsz))
                                kt += gsz
                            for qk, kt0, gsz in qks:
                                _gc[0] += 1
                                dst = ring[:, ds(kt0, gsz), :].rearrange(
                                    "p t n -> p (t n)"
                                )
                                if _gc[0] % 10 in (2, 4, 7, 9):
                                    # fast-exp on the (otherwise idle) DVE:
                                    # e4m3 bit pattern via scaled uint8 cast
                                    nc.vector.tensor_scalar(
                                        dst.bitcast(U8),
                                        qk[:],
                                        A8,
                                        B8,
                                        mybir.AluOpType.mult,
                                        mybir.AluOpType.add,
                                    )
                                else:
                                    nc.scalar.activation(
                                        dst, qk[:], EXP, bias=nbias_sb[:], scale=1.0
                                    )
                                pending.append((ring, kt0, gsz, h, ci, pv, pb))
                            last_chunk = h == HEADS - 1 and ci == NCH - 1
                            if last_chunk:
                                while len(pending) > 5:
                                    pop_pv()
                            elif len(pending) >= PVLAG + 4:
                                while len(pending) > PVLAG - 3:
                                    pop_pv()
                            if g == 4 and deferred_op:
                                emit_outproj(deferred_op.pop(0))
                            for gg in (g, g + 1):
                                if ci == 0 and h == 0 and gg < 7:
                                    if gg == 0:
                                        emit_k(h, 2, 3)
                                    if gg == 2:
                                        emit_k(h, 5, 3)
                                    if 4 * gg + 4 < NKT:
                                        emit_vt4(4 * gg + 4)
                                if ci == 0 and h > 0 and gg == 2:
                                    emit_k(h, 6, 2)
                                if gg == 1 and ci < NCH - 1:
                                    emit_q(h, ci + 1)
                                if ci == NCH - 2 and h < HEADS - 1 and gg == 3:
                                    emit_k(h + 1, 0, 3)
                                if ci == NCH - 1 and h < HEADS - 1:
                                    if gg == 2:
                                        emit_q(h + 1, 0)
                                    elif gg == 3:
                                        emit_k(h + 1, 3, 3)
                while pending:
                    pop_pv()
                while deferred_op:
                    emit_outproj(deferred_op.pop(0))

    nc.finalize()
    return nc


_NC_CACHE = None


def make_in_maps(x, w_qkv, w_out, b_out):
    bf16 = ml_dtypes.bfloat16
    x = np.ascontiguousarray(np.asarray(x, dtype=np.float32)).reshape(4, C, N)
    w_qkv = np.asarray(w_qkv, dtype=np.float32)
    w_out = np.asarray(w_out, dtype=np.float32)
    b_out = np.asarray(b_out, dtype=np.float32)

    wqT = (w_qkv[0:HID] * SCALE).T                              # [256, 128]
    wkT = w_qkv[HID:2 * HID].T                                  # [256, 128]
    # per-head projection weights, head block replicated 4x along columns
    wqrT = np.ascontiguousarray(
        np.concatenate(
            [np.tile(wqT[:, 32 * h:32 * (h + 1)], (1, 4)) for h in range(HEADS)],
            axis=1,
        )
    ).astype(bf16)
    wkrT = np.ascontiguousarray(
        np.concatenate(
            [np.tile(wkT[:, 32 * h:32 * (h + 1)], (1, 4)) for h in range(HEADS)],
            axis=1,
        )
    ).astype(bf16)
    wvT = np.ascontiguousarray(w_qkv[2 * HID:3 * HID].T).astype(bf16)
    woT = np.ascontiguousarray(w_out.T)                         # [128, 256]
    boutc = np.ascontiguousarray(b_out.reshape(C, 1))
    xbf = x.astype(bf16)

    in_maps = []
    for core in range(NCORES):
        b, half = divmod(core, 2)
        in_maps.append(
            {
                "xb": xbf[b],
                "xq": np.ascontiguousarray(xbf[b][:, half * NQ:(half + 1) * NQ]),
                "wqrT": wqrT,
                "wkrT": wkrT,
                "wvT": wvT,
                "woT": woT,
                "bout": boutc,
            }
        )
    return in_maps


def kernel(x, w_qkv, w_out, b_out):
    global _NC_CACHE
    if _NC_CACHE is None:
        _NC_CACHE = build_nc()
    nc = _NC_CACHE
    in_maps = make_in_maps(x, w_qkv, w_out, b_out)
    res = run_bass_kernel_spmd(nc, in_maps, core_ids=list(range(NCORES)))
    out = np.empty((4, C, N), dtype=np.float32)
    for core in range(NCORES):
        b, half = divmod(core, 2)
        out[b][:, half * NQ:(half + 1) * NQ] = res.results[core]["out"]
    return out.reshape(4, C, 64, 64)


# revision 28
# speedup vs baseline: 1.1378x; 1.1378x over previous
"""Trainium2 Bass kernel for nn_Attention_21715354649378.

Reference computation (per batch b of 4):
    qkv = w_qkv @ x        x: [256, 4096(=64x64)]   w_qkv: [384, 256]
    q,k,v: [4 heads, 32, 4096];  q *= 32**-0.5
    sim_h = q_h^T k_h   [4096, 4096];  attn = softmax(sim, axis=-1)
    out_h = attn @ v_h^T    -> [4096, 32]
    out = w_out @ concat_heads + b_out   [256, 4096]

Sharding: 8 cores = 4 batches x 2 query-halves. Each core computes K/V for
its full batch plus attention + output projection for its half of the query
pixels. Outputs are disjoint slices -> no collectives.

Device algorithm per core (keys-in-partition layout; probs are kept in
fp8e4m3 with a global exp-shift of -2 which cancels in the softmax ratio):
    vT8 = x^T W_v^T      fp8 pair layout [128keys, pair, j, (h|1)x48]
    krep_h = repl4(W_k,h) x   [128 = 4 copies of k_h(32d), 4096]  bf16
    qrep_h = repl4(s W_q,h) xq [128, 2048] bf16
    flat software pipeline over chunks (h, ci) and key-tile groups, with two
    alternating PSUM staging pools (4 + 3 banks) shared with the projection
    stream; PV trails exp so activations run back-to-back:
        simT[kt] = krep_h[band, kt].T @ qrep_h[band, ci]   -> PSUM
        probs8 = exp(simT - 2)  -> fp8 ring  (ScalarE activation, or DVE
                 via Schraudolph: uint8(rint(x*a + b)) = e4m3 bits)
        pv += DoubleRow fp8 matmul over key-tile PAIRS:
              [vh|1](128,2,33).T @ probs8(128,2,512)  -> [33, 512]
    rows 0..31 = unnormalized out, row 32 = softmax denominator;
    outh[ci][32h:] = pv[0:32] * bcast(1/pv[32]) (recip + DRAM-bounce DMA)
    out[ci] = W_o @ outh[ci] + b_out  -> DMA out
"""

import numpy as np
import ml_dtypes

import concourse.bass as bass
import concourse.mybir as mybir
import concourse.tile as tile
from concourse import bacc
from concourse.bass import ts, ds
from concourse.bass_utils import run_bass_kernel_spmd

HEADS = 4
D = 32
HID = 128
C = 256
N = 4096
NQ = 2048
SCALE = D ** -0.5
NCORES = 8

F32 = mybir.dt.float32
F32R = mybir.dt.float32r
BF16 = mybir.dt.bfloat16
F8 = mybir.dt.float8e4
U8 = mybir.dt.uint8
EXP = mybir.ActivationFunctionType.Exp
DRMODE = mybir.MatmulPerfMode.DoubleRow

# exp shift: probs = exp(sim - C2); cancels in softmax normalization but
# keeps exp(sim) within fp8e4m3 range (max 240) for sim up to ~7.4
C2 = 2.0
# Schraudolph fp8e4m3: bits = rint(x*A8 + B8) as saturating uint8
A8 = 8.0 / np.log(2.0)
B8 = 56.0 - C2 * A8

NKT = N // 128  # 32 key tiles per chunk
NPAIR = NKT // 2  # 16 DoubleRow pairs
NCH = NQ // 512  # 4 query chunks
PVLAG = 8  # PV trails its exp by this many staging groups
VTW = 48  # padded per-head width in vT8 (33 used; 48 for 16B ldw stride)


def build_nc():
    nc = bacc.Bacc("TRN2")

    xb = nc.declare_dram_parameter("xb", [C, N], BF16, isOutput=False)
    xq = nc.declare_dram_parameter("xq", [C, NQ], BF16, isOutput=False)
    wqrT = nc.declare_dram_parameter("wqrT", [C, HEADS * HID], BF16, isOutput=False)
    wkrT = nc.declare_dram_parameter("wkrT", [C, HEADS * HID], BF16, isOutput=False)
    wvT = nc.declare_dram_parameter("wvT", [C, HID], BF16, isOutput=False)
    woT = nc.declare_dram_parameter("woT", [HID, C], F32R, isOutput=False)
    bout = nc.declare_dram_parameter("bout", [C, 1], F32, isOutput=False)
    out = nc.declare_dram_parameter("out", [C, NQ], F32, isOutput=True)

    with tile.TileContext(nc) as tc:
        with (
            nc.allow_low_precision(reason="bf16/fp8 attention core"),
            tc.tile_pool(name="persist", bufs=1) as persist,
            tc.tile_pool(name="wts", bufs=1) as wts,
            tc.tile_pool(name="dram", bufs=2, space="DRAM") as dram_pool,
        ):
            # ---- persistent SBUF tensors ----
            x_sb = [
                [
                    persist.tile([128, N // 2], BF16, tag=f"x{i}{j}", name=f"x{i}{j}")
                    for j in range(2)
                ]
                for i in range(2)
            ]
            xq_sb = [
                persist.tile([128, NQ], BF16, tag=f"xq{i}", name=f"xq{i}")
                for i in range(2)
            ]
            krep = [
                persist.tile([128, N], BF16, tag=f"krep{h}", name=f"krep{h}")
                for h in range(HEADS)
            ]
            qrep = [
                persist.tile([128, NQ], BF16, tag=f"qrep{h}", name=f"qrep{h}")
                for h in range(HEADS)
            ]
            # fp8 V in DoubleRow pair layout: [128keys, pair, j, h*VTW + m]
            # (m: 0..31 = v dims, 32 = ones for the softmax denominator)
            vt8 = persist.tile(
                [128, NPAIR, 2, HEADS * VTW], F8, tag="vt8", name="vt8"
            )

            wqr_sb = [
                wts.tile([128, HEADS * HID], BF16, tag=f"wqr{i}", name=f"wqr{i}")
                for i in range(2)
            ]
            wkr_sb = [
                wts.tile([128, HEADS * HID], BF16, tag=f"wkr{i}", name=f"wkr{i}")
                for i in range(2)
            ]
            wv_sb = [
                wts.tile([128, HID], BF16, tag=f"wv{i}", name=f"wv{i}")
                for i in range(2)
            ]
            wo_sb = wts.tile([HID, C], F32R, tag="wo")
            bo_sb = [
                wts.tile([128, 1], F32, tag=f"bo{i}", name=f"bo{i}")
                for i in range(2)
            ]
            ones_sb = wts.tile([1, D], F32, tag="ones")
            nbias_sb = wts.tile([128, 1], F32, tag="nbias")

            # ---- DMA inputs, ordered by first use (~0.6us issue each) ----
            for i in range(2):
                nc.sync.dma_start(out=wkr_sb[i][:], in_=wkrT[ds(i * 128, 128), :])
            for i in range(2):
                nc.sync.dma_start(
                    out=x_sb[i][0][:], in_=xb[ds(i * 128, 128), ts(0, N // 2)]
                )
            for i in range(2):
                nc.gpsimd.dma_start(out=wv_sb[i][:], in_=wvT[ds(i * 128, 128), :])
                nc.gpsimd.dma_start(out=wqr_sb[i][:], in_=wqrT[ds(i * 128, 128), :])
            for i in range(2):
                nc.gpsimd.dma_start(out=xq_sb[i][:], in_=xq[ds(i * 128, 128), :])
            for i in range(2):
                nc.sync.dma_start(
                    out=x_sb[i][1][:], in_=xb[ds(i * 128, 128), ts(1, N // 2)]
                )
                nc.gpsimd.dma_start(out=bo_sb[i][:], in_=bout[ds(i * 128, 128), :])
            nc.gpsimd.dma_start(out=wo_sb[:], in_=woT[:, :])
            nc.vector.memset(vt8[:], 1.0)
            nc.vector.memset(ones_sb[:], 1.0)
            nc.vector.memset(nbias_sb[:], -C2)

            with (
                tc.tile_pool(name="qkA", bufs=1, space="PSUM") as qkA,
                tc.tile_pool(name="qkB", bufs=1, space="PSUM") as qkB,
                tc.tile_pool(name="pvp", bufs=1, space="PSUM") as pvp,
                tc.tile_pool(name="ring", bufs=2) as ring_pool,
                tc.tile_pool(name="norm", bufs=3) as norm_pool,
                tc.tile_pool(name="osb", bufs=2) as osb,
            ):
                # staging slots rotate globally between the two pools;
                # projection tiles share the same rotation (no extra banks)
                _ptog = [0]

                def x_ap(ct, c0, length):
                    t_idx = c0 // (N // 2)
                    return x_sb[ct][t_idx][:, ds(c0 % (N // 2), length)]

                def xq_ap(ct, c0, length):
                    return xq_sb[ct][:, ds(c0, length)]

                def next_pool():
                    pool = qkA if _ptog[0] == 0 else qkB
                    _ptog[0] ^= 1
                    return pool

                # greedy engine-load leveling for elementwise PSUM work:
                # route each exp / evacuation op to whichever of ScalarE /
                # VectorE has the lower accumulated busy estimate
                est = {"S": 0.0, "V": 0.0}

                def s_cost(fd):
                    return (172 + fd) / 1.2 * 1.15

                def v_cost(fd):
                    return (120 + fd) / 0.96 * 1.15

                def routed_copy(dst, src_ap, fd):
                    if est["S"] + s_cost(fd) <= est["V"] + v_cost(fd):
                        est["S"] += s_cost(fd)
                        nc.scalar.copy(dst, src_ap)
                    else:
                        est["V"] += v_cost(fd)
                        nc.vector.tensor_copy(dst, src_ap)

                def routed_exp(dst, qk, fd):
                    if est["S"] + s_cost(fd) <= est["V"] + v_cost(fd):
                        est["S"] += s_cost(fd)
                        nc.scalar.activation(
                            dst, qk, EXP, bias=nbias_sb[:], scale=1.0
                        )
                    else:
                        est["V"] += v_cost(fd)
                        nc.vector.tensor_scalar(
                            dst.bitcast(U8),
                            qk,
                            A8,
                            B8,
                            mybir.AluOpType.mult,
                            mybir.AluOpType.add,
                        )

                def proj_tile(cols):
                    pool = next_pool()
                    t = pool.tile(
                        [128, (4 if pool is qkA else 3) * 512],
                        F32,
                        tag="qk",
                        name="ps",
                    )
                    return t[:, 0:cols]

                def emit_vt4(kt0):
                    # four key tiles' vT in one staging slot, one strided copy
                    ps = proj_tile(4 * HID)
                    for t in range(4):
                        for ct in range(2):
                            nc.tensor.matmul(
                                ps[:, ts(t, HID)],
                                x_ap(ct, (kt0 + t) * 128, 128),
                                wv_sb[ct][:],
                                start=(ct == 0),
                                stop=(ct == 1),
                            )
                    # dst: [p, pair(2), j(2), h(4), m(32)] slice of vt8
                    dst = vt8[:, ds(kt0 // 2, 2), :, :].rearrange(
                        "p t j (h m) -> p t j h m", m=VTW
                    )[:, :, :, :, 0:32]
                    src = ps.rearrange("p (t h m) -> p t h m", t=4, m=32).rearrange(
                        "p (tp j) h m -> p tp j h m", j=2
                    )
                    routed_copy(dst, src, 4 * HID)

                def emit_k(h, j, nj=1):
                    # fill one staging slot with nj 512-col key-projection
                    # blocks (one matmul pair per block, a single evac CAST)
                    ps = proj_tile(nj * 512)
                    for b in range(nj):
                        for ct in range(2):
                            nc.tensor.matmul(
                                ps[:, ts(b, 512)],
                                wkr_sb[ct][:, ts(h, HID)],
                                x_ap(ct, (j + b) * 512, 512),
                                start=(ct == 0),
                                stop=(ct == 1),
                            )
                    routed_copy(krep[h][:, ds(j * 512, nj * 512)], ps[:], nj * 512)

                def emit_q(h, j, nj=1):
                    ps = proj_tile(nj * 512)
                    for b in range(nj):
                        for ct in range(2):
                            nc.tensor.matmul(
                                ps[:, ts(b, 512)],
                                wqr_sb[ct][:, ts(h, HID)],
                                xq_ap(ct, (j + b) * 512, 512),
                                start=(ct == 0),
                                stop=(ct == 1),
                            )
                    routed_copy(qrep[h][:, ds(j * 512, nj * 512)], ps[:], nj * 512)

                outh = [
                    osb.tile([HID, 512], F32R, tag=f"outh{c}", name=f"outh{c}")
                    for c in range(NCH)
                ]

                # one PSUM bank holds TWO pv accumulators (partitions 0 and
                # 64), alternating per (h,ci) so a chunk's first PV matmul
                # never stalls the in-order Tensor queue waiting for the
                # previous chunk's normalize to evacuate the bank.
                pv_bank = pvp.tile([128, 512], F32, tag="pv", name="pv")

                def emit_norm(h, ci, pv, tail=False):
                    est["V"] += v_cost(512) * 2 + 700.0
                    # rows 0..31 / row 32
                    pvs = norm_pool.tile([33, 512], F32, tag="pvs", name="pvs")
                    nc.vector.tensor_copy(pvs[:], pv[0:33, :])
                    den = norm_pool.tile([1, 512], F32, tag="den", name="den")
                    nc.vector.tensor_copy(den[:], pv[32:33, :])
                    rec = norm_pool.tile([1, 512], F32, tag="rec", name="rec")
                    # (reciprocal_approx_fast requires a partition-0 source)
                    nc.vector.reciprocal_approx_fast(rec[:], den[:])
                    if tail:
                        # staging slots are free at the end: matmul-broadcast
                        # avoids the ~5us DRAM round-trip on the critical tail
                        bcp = next_pool()
                        bct = bcp.tile(
                            [128, (4 if bcp is qkA else 3) * 512],
                            F32,
                            tag="qk",
                            name="bct",
                        )
                        nc.tensor.matmul(
                            bct[0:D, 0:512],
                            ones_sb[:],
                            rec[:],
                            start=True,
                            stop=True,
                        )
                        nc.vector.tensor_mul(
                            outh[ci][ds(32 * h, 32), :],
                            pvs[0:32, :],
                            bct[0:D, 0:512],
                        )
                        return
                    # broadcast 1/denom to 32 partitions via DRAM bounce
                    rdr = dram_pool.tile([1, 512], F32, tag="rdr", name="rdr")
                    nc.sync.dma_start(out=rdr[:], in_=rec[:])
                    bc = norm_pool.tile([D, 512], F32, tag="bc", name="bc")
                    nc.sync.dma_start(
                        out=bc[:],
                        in_=bass.AP(
                            tensor=rdr.tensor,
                            offset=rdr.offset,
                            ap=[[0, D]] + [list(a) for a in rdr.ap[1:]],
                        ),
                    )
                    nc.gpsimd.tensor_mul(
                        outh[ci][ds(32 * h, 32), :], pvs[0:32, :], bc[:]
                    )

                pending = []
                deferred_op = []
                _gc = [0]

                def emit_outproj(ci):
                    for ot in range(2):
                        op = proj_tile(512)
                        nc.tensor.matmul(
                            op,
                            wo_sb[:, ts(ot, 128)],
                            outh[ci][:],
                            start=True,
                            stop=True,
                        )
                        ob = osb.tile([128, 512], F32, tag="ob", name="ob")
                        est["V"] += v_cost(512)
                        nc.vector.tensor_scalar_add(ob[:], op, bo_sb[ot][:])
                        nc.sync.dma_start(
                            out=out[ds(ot * 128, 128), ts(ci, 512)], in_=ob[:]
                        )

                # per-(h,ci) watermark of PV'd key tiles (pairwise DoubleRow)
                pv_done = {}

                def pop_pv():
                    ring, kt0, gsz, h, ci, pv, pb = pending.pop(0)
                    wm = kt0 + gsz
                    done = pv_done.get((h, ci), 0)
                    while done + 2 <= wm:
                        t = done // 2
                        lhsT = vt8[:, ds(t, 1), :, ds(h * VTW, 33)].rearrange(
                            "p a j m -> p (a j) m"
                        )
                        rhs = ring[:, ds(2 * t, 2), :]
                        nc.tensor.matmul(
                            pv,
                            lhsT,
                            rhs,
                            start=(t == 0),
                            stop=(t == NPAIR - 1),
                            perf_mode=DRMODE,
                        )
                        done += 2
                    pv_done[(h, ci)] = done
                    if done == NKT:
                        last = h == HEADS - 1 and ci == NCH - 1
                        emit_norm(h, ci, pv, tail=last)
                        if h == HEADS - 1:
                            deferred_op.append(ci)

                # prologue: first projections
                emit_k(0, 0, 2)
                emit_vt4(0)
                emit_q(0, 0)

                for h in range(HEADS):
                    for ci in range(NCH):
                        pb = 0
                        pv = pv_bank[ds(pb, 33), :]
                        ring = ring_pool.tile(
                            [128, NKT, 512], F8, tag="ring", name="ring"
                        )
                        kt = 0
                        g = -2
                        while kt < NKT:
                            g += 2
                            # pair of QK groups back-to-back: a full<->tiled
                            # PE mode switch drains the array, so batching
                            # two row-banded QK groups (then two PV groups)
                            # halves the switches and keeps QKs concurrent
                            qks = []
                            for _ in range(2):
                                if kt >= NKT:
                                    break
                                pool = next_pool()
                                gsz = min(4 if pool is qkA else 3, NKT - kt)
                                qk = pool.tile(
                                    [128, gsz * 512], F32, tag="qk", name="qkg"
                                )
                                for j in range(gsz):
                                    band = (kt + j) % 4
                                    nc.tensor.matmul(
                                        qk[:, ts(j, 512)],
                                        krep[h][ds(32 * band, 32), ts(kt + j, 128)],
                                        qrep[h][ds(32 * band, 32), ts(ci, 512)],
                                        start=True,
                                        stop=True,
                                        tile_position=(32 * band, 0),
                                    )
                                qks.append((qk, kt, gsz))
                                kt += gsz
                            for qk, kt0, gsz in qks:
                                _gc[0] += 1
                                dst = ring[:, ds(kt0, gsz), :].rearrange(
                                    "p t n -> p (t n)"
                                )
                                routed_exp(dst, qk[:], gsz * 512)
                                pending.append((ring, kt0, gsz, h, ci, pv, pb))
                            while len(pending) > PVLAG:
                                pop_pv()
                            if g == 4 and deferred_op:
                                emit_outproj(deferred_op.pop(0))
                            for gg in (g, g + 1):
                                if ci == 0 and h == 0 and gg < 7:
                                    if gg == 0:
                                        emit_k(h, 2, 3)
                                    if gg == 2:
                                        emit_k(h, 5, 3)
                                    if 4 * gg + 4 < NKT:
                                        emit_vt4(4 * gg + 4)
                                if ci == 0 and h > 0 and gg == 2:
                                    emit_k(h, 6, 2)
                                if gg == 1 and ci < NCH - 1:
                                    emit_q(h, ci + 1)
                                if ci == NCH - 2 and h < HEADS - 1 and gg == 3:
                                    emit_k(h + 1, 0, 3)
                                if ci == NCH - 1 and h < HEADS - 1:
                                    if gg == 2:
                                        emit_q(h + 1, 0)
                                    elif gg == 3:
                                        emit_k(h + 1, 3, 3)
                while pending:
                    pop_pv()
                while deferred_op:
                    emit_outproj(deferred_op.pop(0))

    nc.finalize()
    return nc


_NC_CACHE = None


def make_in_maps(x, w_qkv, w_out, b_out):
    bf16 = ml_dtypes.bfloat16
    x = np.ascontiguousarray(np.asarray(x, dtype=np.float32)).reshape(4, C, N)
    w_qkv = np.asarray(w_qkv, dtype=np.float32)
    w_out = np.asarray(w_out, dtype=np.float32)
    b_out = np.asarray(b_out, dtype=np.float32)

    wqT = (w_qkv[0:HID] * SCALE).T                              # [256, 128]
    wkT = w_qkv[HID:2 * HID].T                                  # [256, 128]
    # per-head projection weights, head block replicated 4x along columns
    wqrT = np.ascontiguousarray(
        np.concatenate(
            [np.tile(wqT[:, 32 * h:32 * (h + 1)], (1, 4)) for h in range(HEADS)],
            axis=1,
        )
    ).astype(bf16)
    wkrT = np.ascontiguousarray(
        np.concatenate(
            [np.tile(wkT[:, 32 * h:32 * (h + 1)], (1, 4)) for h in range(HEADS)],
            axis=1,
        )
    ).astype(bf16)
    wvT = np.ascontiguousarray(w_qkv[2 * HID:3 * HID].T).astype(bf16)
    woT = np.ascontiguousarray(w_out.T)                         # [128, 256]
    boutc = np.ascontiguousarray(b_out.reshape(C, 1))
    xbf = x.astype(bf16)

    in_maps = []
    for core in range(NCORES):
        b, half = divmod(core, 2)
        in_maps.append(
            {
                "xb": xbf[b],
                "xq": np.ascontiguousarray(xbf[b][:, half * NQ:(half + 1) * NQ]),
                "wqrT": wqrT,
                "wkrT": wkrT,
                "wvT": wvT,
                "woT": woT,
                "bout": boutc,
            }
        )
    return in_maps


def kernel(x, w_qkv, w_out, b_out):
    global _NC_CACHE
    if _NC_CACHE is None:
        _NC_CACHE = build_nc()
    nc = _NC_CACHE
    in_maps = make_in_maps(x, w_qkv, w_out, b_out)
    res = run_bass_kernel_spmd(nc, in_maps, core_ids=list(range(NCORES)))
    out = np.empty((4, C, N), dtype=np.float32)
    for core in range(NCORES):
        b, half = divmod(core, 2)
        out[b][:, half * NQ:(half + 1) * NQ] = res.results[core]["out"]
    return out.reshape(4, C, 64, 64)


# revision 29
# speedup vs baseline: 1.1797x; 1.0368x over previous
"""Trainium2 Bass kernel for nn_Attention_21715354649378.

Reference computation (per batch b of 4):
    qkv = w_qkv @ x        x: [256, 4096(=64x64)]   w_qkv: [384, 256]
    q,k,v: [4 heads, 32, 4096];  q *= 32**-0.5
    sim_h = q_h^T k_h   [4096, 4096];  attn = softmax(sim, axis=-1)
    out_h = attn @ v_h^T    -> [4096, 32]
    out = w_out @ concat_heads + b_out   [256, 4096]

Sharding: 8 cores = 4 batches x 2 query-halves. Each core computes K/V for
its full batch plus attention + output projection for its half of the query
pixels. Outputs are disjoint slices -> no collectives.

Device algorithm per core (keys-in-partition layout; probs are kept in
fp8e4m3 with a global exp-shift of -2 which cancels in the softmax ratio):
    vT8 = x^T W_v^T      fp8 pair layout [128keys, pair, j, (h|1)x48]
    krep_h = repl4(W_k,h) x   [128 = 4 copies of k_h(32d), 4096]  bf16
    qrep_h = repl4(s W_q,h) xq [128, 2048] bf16
    flat software pipeline over chunks (h, ci) and key-tile groups, with two
    alternating PSUM staging pools (4 + 3 banks) shared with the projection
    stream; PV trails exp so activations run back-to-back:
        simT[kt] = krep_h[band, kt].T @ qrep_h[band, ci]   -> PSUM
        probs8 = exp(simT - 2)  -> fp8 ring  (ScalarE activation, or DVE
                 via Schraudolph: uint8(rint(x*a + b)) = e4m3 bits)
        pv += DoubleRow fp8 matmul over key-tile PAIRS:
              [vh|1](128,2,33).T @ probs8(128,2,512)  -> [33, 512]
    rows 0..31 = unnormalized out, row 32 = softmax denominator;
    outh[ci][32h:] = pv[0:32] * bcast(1/pv[32]) (recip + DRAM-bounce DMA)
    out[ci] = W_o @ outh[ci] + b_out  -> DMA out
"""

import numpy as np
import ml_dtypes

import concourse.bass as bass
import concourse.mybir as mybir
import concourse.tile as tile
from concourse import bacc
from concourse.bass import ts, ds
from concourse.bass_utils import run_bass_kernel_spmd

HEADS = 4
D = 32
HID = 128
C = 256
N = 4096
NQ = 2048
SCALE = D ** -0.5
NCORES = 8

F32 = mybir.dt.float32
F32R = mybir.dt.float32r
BF16 = mybir.dt.bfloat16
F8 = mybir.dt.float8e4
U8 = mybir.dt.uint8
EXP = mybir.ActivationFunctionType.Exp
DRMODE = mybir.MatmulPerfMode.DoubleRow

# exp shift: probs = exp(sim - C2); cancels in softmax normalization but
# keeps exp(sim) within fp8e4m3 range (max 240) for sim up to ~7.4
C2 = 2.0
# Schraudolph fp8e4m3: bits = rint(x*A8 + B8) as saturating uint8
A8 = 8.0 / np.log(2.0)
B8 = 56.0 - C2 * A8

NKT = N // 128  # 32 key tiles per chunk
NPAIR = NKT // 2  # 16 DoubleRow pairs
NCH = NQ // 512  # 4 query chunks
PVLAG = 8  # PV trails its exp by this many staging groups
VTW = 48  # padded per-head width in vT8 (33 used; 48 for 16B ldw stride)


def build_nc():
    nc = bacc.Bacc("TRN2")

    xb = nc.declare_dram_parameter("xb", [C, N], BF16, isOutput=False)
    xq = nc.declare_dram_parameter("xq", [C, NQ], BF16, isOutput=False)
    wqrT = nc.declare_dram_parameter("wqrT", [C, HEADS * HID], BF16, isOutput=False)
    wkrT = nc.declare_dram_parameter("wkrT", [C, HEADS * HID], BF16, isOutput=False)
    wvT = nc.declare_dram_parameter("wvT", [C, HID], BF16, isOutput=False)
    woT = nc.declare_dram_parameter("woT", [HID, C], F32R, isOutput=False)
    bout = nc.declare_dram_parameter("bout", [C, 1], F32, isOutput=False)
    out = nc.declare_dram_parameter("out", [C, NQ], F32, isOutput=True)

    with tile.TileContext(nc) as tc:
        with (
            nc.allow_low_precision(reason="bf16/fp8 attention core"),
            tc.tile_pool(name="persist", bufs=1) as persist,
            tc.tile_pool(name="wts", bufs=1) as wts,
            tc.tile_pool(name="dram", bufs=2, space="DRAM") as dram_pool,
        ):
            # ---- persistent SBUF tensors ----
            x_sb = [
                [
                    persist.tile([128, N // 2], BF16, tag=f"x{i}{j}", name=f"x{i}{j}")
                    for j in range(2)
                ]
                for i in range(2)
            ]
            xq_sb = [
                persist.tile([128, NQ], BF16, tag=f"xq{i}", name=f"xq{i}")
                for i in range(2)
            ]
            krep = [
                persist.tile([128, N], BF16, tag=f"krep{h}", name=f"krep{h}")
                for h in range(HEADS)
            ]
            qrep = [
                persist.tile([128, NQ], BF16, tag=f"qrep{h}", name=f"qrep{h}")
                for h in range(HEADS)
            ]
            # fp8 V in DoubleRow pair layout: [128keys, pair, j, h*VTW + m]
            # (m: 0..31 = v dims, 32 = ones for the softmax denominator)
            vt8 = persist.tile(
                [128, NPAIR, 2, HEADS * VTW], F8, tag="vt8", name="vt8"
            )

            wqr_sb = [
                wts.tile([128, HEADS * HID], BF16, tag=f"wqr{i}", name=f"wqr{i}")
                for i in range(2)
            ]
            wkr_sb = [
                wts.tile([128, HEADS * HID], BF16, tag=f"wkr{i}", name=f"wkr{i}")
                for i in range(2)
            ]
            wv_sb = [
                wts.tile([128, HID], BF16, tag=f"wv{i}", name=f"wv{i}")
                for i in range(2)
            ]
            wo_sb = wts.tile([HID, C], F32R, tag="wo")
            bo_sb = [
                wts.tile([128, 1], F32, tag=f"bo{i}", name=f"bo{i}")
                for i in range(2)
            ]
            ones_sb = wts.tile([1, D], F32, tag="ones")
            nbias_sb = wts.tile([128, 1], F32, tag="nbias")

            # ---- DMA inputs, ordered by first use (~0.6us issue each) ----
            for i in range(2):
                nc.sync.dma_start(out=wkr_sb[i][:], in_=wkrT[ds(i * 128, 128), :])
            for i in range(2):
                nc.sync.dma_start(
                    out=x_sb[i][0][:], in_=xb[ds(i * 128, 128), ts(0, N // 2)]
                )
            for i in range(2):
                nc.sync.dma_start(out=wv_sb[i][:], in_=wvT[ds(i * 128, 128), :])
                nc.sync.dma_start(out=wqr_sb[i][:], in_=wqrT[ds(i * 128, 128), :])
            for i in range(2):
                nc.sync.dma_start(out=xq_sb[i][:], in_=xq[ds(i * 128, 128), :])
            for i in range(2):
                nc.sync.dma_start(
                    out=x_sb[i][1][:], in_=xb[ds(i * 128, 128), ts(1, N // 2)]
                )
                nc.sync.dma_start(out=bo_sb[i][:], in_=bout[ds(i * 128, 128), :])
            nc.sync.dma_start(out=wo_sb[:], in_=woT[:, :])
            nc.vector.memset(vt8[:], 1.0)
            nc.vector.memset(ones_sb[:], 1.0)
            nc.vector.memset(nbias_sb[:], -C2)

            with (
                tc.tile_pool(name="qkA", bufs=1, space="PSUM") as qkA,
                tc.tile_pool(name="qkB", bufs=1, space="PSUM") as qkB,
                tc.tile_pool(name="pvp", bufs=1, space="PSUM") as pvp,
                tc.tile_pool(name="ring", bufs=2) as ring_pool,
                tc.tile_pool(name="norm", bufs=3) as norm_pool,
                tc.tile_pool(name="osb", bufs=2) as osb,
            ):
                # staging slots rotate globally between the two pools;
                # projection tiles share the same rotation (no extra banks)
                _ptog = [0]

                def x_ap(ct, c0, length):
                    t_idx = c0 // (N // 2)
                    return x_sb[ct][t_idx][:, ds(c0 % (N // 2), length)]

                def xq_ap(ct, c0, length):
                    return xq_sb[ct][:, ds(c0, length)]

                def next_pool():
                    pool = qkA if _ptog[0] == 0 else qkB
                    _ptog[0] ^= 1
                    return pool

                # greedy engine-load leveling for elementwise PSUM work:
                # route each exp / evacuation op to whichever of ScalarE /
                # VectorE has the lower accumulated busy estimate
                est = {"S": 0.0, "V": 0.0}

                def s_cost(fd):
                    return (172 + fd) / 1.2 * 1.15

                def v_cost(fd):
                    return (120 + fd) / 0.96 * 1.15

                def routed_copy(dst, src_ap, fd):
                    nc.vector.tensor_copy(dst, src_ap)

                def routed_exp(dst, qk, fd):
                    _gc[0] += 1
                    if _gc[0] % 10 in (3, 6, 9):
                        nc.vector.tensor_scalar(
                            dst.bitcast(U8),
                            qk,
                            A8,
                            B8,
                            mybir.AluOpType.mult,
                            mybir.AluOpType.add,
                        )
                    else:
                        nc.scalar.activation(
                            dst, qk, EXP, bias=nbias_sb[:], scale=1.0
                        )

                def proj_tile(cols):
                    pool = next_pool()
                    t = pool.tile(
                        [128, (4 if pool is qkA else 3) * 512],
                        F32,
                        tag="qk",
                        name="ps",
                    )
                    return t[:, 0:cols]

                def emit_vt4(kt0):
                    # four key tiles' vT in one staging slot, one strided copy
                    ps = proj_tile(4 * HID)
                    for t in range(4):
                        for ct in range(2):
                            nc.tensor.matmul(
                                ps[:, ts(t, HID)],
                                x_ap(ct, (kt0 + t) * 128, 128),
                                wv_sb[ct][:],
                                start=(ct == 0),
                                stop=(ct == 1),
                            )
                    # dst: [p, pair(2), j(2), h(4), m(32)] slice of vt8
                    dst = vt8[:, ds(kt0 // 2, 2), :, :].rearrange(
                        "p t j (h m) -> p t j h m", m=VTW
                    )[:, :, :, :, 0:32]
                    src = ps.rearrange("p (t h m) -> p t h m", t=4, m=32).rearrange(
                        "p (tp j) h m -> p tp j h m", j=2
                    )
                    routed_copy(dst, src, 4 * HID)

                def emit_k(h, j, nj=1):
                    # fill one staging slot with nj 512-col key-projection
                    # blocks (one matmul pair per block, a single evac CAST)
                    ps = proj_tile(nj * 512)
                    for b in range(nj):
                        for ct in range(2):
                            nc.tensor.matmul(
                                ps[:, ts(b, 512)],
                                wkr_sb[ct][:, ts(h, HID)],
                                x_ap(ct, (j + b) * 512, 512),
                                start=(ct == 0),
                                stop=(ct == 1),
                            )
                    routed_copy(krep[h][:, ds(j * 512, nj * 512)], ps[:], nj * 512)

                def emit_q(h, j, nj=1):
                    ps = proj_tile(nj * 512)
                    for b in range(nj):
                        for ct in range(2):
                            nc.tensor.matmul(
                                ps[:, ts(b, 512)],
                                wqr_sb[ct][:, ts(h, HID)],
                                xq_ap(ct, (j + b) * 512, 512),
                                start=(ct == 0),
                                stop=(ct == 1),
                            )
                    routed_copy(qrep[h][:, ds(j * 512, nj * 512)], ps[:], nj * 512)

                outh = [
                    osb.tile([HID, 512], F32R, tag=f"outh{c}", name=f"outh{c}")
                    for c in range(NCH)
                ]

                # one PSUM bank holds TWO pv accumulators (partitions 0 and
                # 64), alternating per (h,ci) so a chunk's first PV matmul
                # never stalls the in-order Tensor queue waiting for the
                # previous chunk's normalize to evacuate the bank.
                pv_bank = pvp.tile([128, 512], F32, tag="pv", name="pv")

                def emit_norm(h, ci, pv, tail=False):
                    est["V"] += v_cost(512) * 2 + 700.0
                    # rows 0..31 / row 32
                    pvs = norm_pool.tile([33, 512], F32, tag="pvs", name="pvs")
                    nc.vector.tensor_copy(pvs[:], pv[0:33, :])
                    den = norm_pool.tile([1, 512], F32, tag="den", name="den")
                    nc.vector.tensor_copy(den[:], pv[32:33, :])
                    rec = norm_pool.tile([1, 512], F32, tag="rec", name="rec")
                    # (reciprocal_approx_fast requires a partition-0 source)
                    nc.vector.reciprocal_approx_fast(rec[:], den[:])
                    if tail:
                        # staging slots are free at the end: matmul-broadcast
                        # avoids the ~5us DRAM round-trip on the critical tail
                        bcp = next_pool()
                        bct = bcp.tile(
                            [128, (4 if bcp is qkA else 3) * 512],
                            F32,
                            tag="qk",
                            name="bct",
                        )
                        nc.tensor.matmul(
                            bct[0:D, 0:512],
                            ones_sb[:],
                            rec[:],
                            start=True,
                            stop=True,
                        )
                        nc.vector.tensor_mul(
                            outh[ci][ds(32 * h, 32), :],
                            pvs[0:32, :],
                            bct[0:D, 0:512],
                        )
                        return
                    # broadcast 1/denom to 32 partitions via DRAM bounce
                    rdr = dram_pool.tile([1, 512], F32, tag="rdr", name="rdr")
                    nc.sync.dma_start(out=rdr[:], in_=rec[:])
                    bc = norm_pool.tile([D, 512], F32, tag="bc", name="bc")
                    nc.sync.dma_start(
                        out=bc[:],
                        in_=bass.AP(
                            tensor=rdr.tensor,
                            offset=rdr.offset,
                            ap=[[0, D]] + [list(a) for a in rdr.ap[1:]],
                        ),
                    )
                    nc.gpsimd.tensor_mul(
                        outh[ci][ds(32 * h, 32), :], pvs[0:32, :], bc[:]
                    )

                pending = []
                deferred_op = []
                _gc = [0]

                def emit_outproj(ci):
                    for ot in range(2):
                        op = proj_tile(512)
                        nc.tensor.matmul(
                            op,
                            wo_sb[:, ts(ot, 128)],
                            outh[ci][:],
                            start=True,
                            stop=True,
                        )
                        ob = osb.tile([128, 512], F32, tag="ob", name="ob")
                        est["V"] += v_cost(512)
                        nc.vector.tensor_scalar_add(ob[:], op, bo_sb[ot][:])
                        nc.sync.dma_start(
                            out=out[ds(ot * 128, 128), ts(ci, 512)], in_=ob[:]
                        )

                # per-(h,ci) watermark of PV'd key tiles (pairwise DoubleRow)
                pv_done = {}

                def pop_pv():
                    ring, kt0, gsz, h, ci, pv, pb = pending.pop(0)
                    wm = kt0 + gsz
                    done = pv_done.get((h, ci), 0)
                    while done + 2 <= wm:
                        t = done // 2
                        lhsT = vt8[:, ds(t, 1), :, ds(h * VTW, 33)].rearrange(
                            "p a j m -> p (a j) m"
                        )
                        rhs = ring[:, ds(2 * t, 2), :]
                        nc.tensor.matmul(
                            pv,
                            lhsT,
                            rhs,
                            start=(t == 0),
                            stop=(t == NPAIR - 1),
                            perf_mode=DRMODE,
                        )
                        done += 2
                    pv_done[(h, ci)] = done
                    if done == NKT:
                        last = h == HEADS - 1 and ci == NCH - 1
                        emit_norm(h, ci, pv, tail=last)
                        if h == HEADS - 1:
                            deferred_op.append(ci)

                # prologue: first projections
                emit_k(0, 0, 2)
                emit_vt4(0)
                emit_q(0, 0)

                for h in range(HEADS):
                    for ci in range(NCH):
                        pb = 0
                        pv = pv_bank[ds(pb, 33), :]
                        ring = ring_pool.tile(
                            [128, NKT, 512], F8, tag="ring", name="ring"
                        )
                        kt = 0
                        g = -2
                        while kt < NKT:
                            g += 2
                            # pair of QK groups back-to-back: a full<->tiled
                            # PE mode switch drains the array, so batching
                            # two row-banded QK groups (then two PV groups)
                            # halves the switches and keeps QKs concurrent
                            qks = []
                            for _ in range(2):
                                if kt >= NKT:
                                    break
                                pool = next_pool()
                                gsz = min(4 if pool is qkA else 3, NKT - kt)
                                qk = pool.tile(
                                    [128, gsz * 512], F32, tag="qk", name="qkg"
                                )
                                for j in range(gsz):
                                    band = (kt + j) % 4
                                    nc.tensor.matmul(
                                        qk[:, ts(j, 512)],
                                        krep[h][ds(32 * band, 32), ts(kt + j, 128)],
                                        qrep[h][ds(32 * band, 32), ts(ci, 512)],
                                        start=True,
                                        stop=True,
                                        tile_position=(32 * band, 0),
                                    )
                                qks.append((qk, kt, gsz))
                                kt += gsz
                            for qk, kt0, gsz in qks:
                                dst = ring[:, ds(kt0, gsz), :].rearrange(
                                    "p t n -> p (t n)"
                                )
                                routed_exp(dst, qk[:], gsz * 512)
                                pending.append((ring, kt0, gsz, h, ci, pv, pb))
                            while len(pending) > PVLAG:
                                pop_pv()
                            if g == 4 and deferred_op:
                                emit_outproj(deferred_op.pop(0))
                            for gg in (g, g + 1):
                                if ci == 0 and h == 0 and gg < 7:
                                    if gg == 0:
                                        emit_k(h, 2, 3)
                                    if gg == 2:
                                        emit_k(h, 5, 3)
                                    if 4 * gg + 4 < NKT:
                                        emit_vt4(4 * gg + 4)
                                if ci == 0 and h > 0 and gg == 2:
                                    emit_k(h, 6, 2)
                                if gg == 1 and ci < NCH - 1:
                                    emit_q(h, ci + 1)
                                if ci == NCH - 1 and h < HEADS - 1:
                                    if gg == 2:
                                        emit_q(h + 1, 0)
                                    elif gg == 3:
                                        emit_k(h + 1, 0, 3)
                                    elif gg == 5:
                                        emit_k(h + 1, 3, 3)
                while pending:
                    pop_pv()
                while deferred_op:
                    emit_outproj(deferred_op.pop(0))

    nc.finalize()
    return nc


_NC_CACHE = None


def make_in_maps(x, w_qkv, w_out, b_out):
    bf16 = ml_dtypes.bfloat16
    x = np.ascontiguousarray(np.asarray(x, dtype=np.float32)).reshape(4, C, N)
    w_qkv = np.asarray(w_qkv, dtype=np.float32)
    w_out = np.asarray(w_out, dtype=np.float32)
    b_out = np.asarray(b_out, dtype=np.float32)

    wqT = (w_qkv[0:HID] * SCALE).T                              # [256, 128]
    wkT = w_qkv[HID:2 * HID].T                                  # [256, 128]
    # per-head projection weights, head block replicated 4x along columns
    wqrT = np.ascontiguousarray(
        np.concatenate(
            [np.tile(wqT[:, 32 * h:32 * (h + 1)], (1, 4)) for h in range(HEADS)],
            axis=1,
        )
    ).astype(bf16)
    wkrT = np.ascontiguousarray(
        np.concatenate(
            [np.tile(wkT[:, 32 * h:32 * (h + 1)], (1, 4)) for h in range(HEADS)],
            axis=1,
        )
    ).astype(bf16)
    wvT = np.ascontiguousarray(w_qkv[2 * HID:3 * HID].T).astype(bf16)
    woT = np.ascontiguousarray(w_out.T)                         # [128, 256]
    boutc = np.ascontiguousarray(b_out.reshape(C, 1))
    xbf = x.astype(bf16)

    in_maps = []
    for core in range(NCORES):
        b, half = divmod(core, 2)
        in_maps.append(
            {
                "xb": xbf[b],
                "xq": np.ascontiguousarray(xbf[b][:, half * NQ:(half + 1) * NQ]),
                "wqrT": wqrT,
                "wkrT": wkrT,
                "wvT": wvT,
                "woT": woT,
                "bout": boutc,
            }
        )
    return in_maps


def kernel(x, w_qkv, w_out, b_out):
    global _NC_CACHE
    if _NC_CACHE is None:
        _NC_CACHE = build_nc()
    nc = _NC_CACHE
    in_maps = make_in_maps(x, w_qkv, w_out, b_out)
    res = run_bass_kernel_spmd(nc, in_maps, core_ids=list(range(NCORES)))
    out = np.empty((4, C, N), dtype=np.float32)
    for core in range(NCORES):
        b, half = divmod(core, 2)
        out[b][:, half * NQ:(half + 1) * NQ] = res.results[core]["out"]
    return out.reshape(4, C, 64, 64)
